# revision 49
# baseline (speedup 1.0000x reference)
"""Trainium2 Bass kernel for a dense transformer block (pre-LN, causal MHA + GELU MLP).

Reference computation (per batch element b, all fp32):
    h   = LN(x; ln1_g, ln1_b)
    q,k,v = h @ wq, h @ wk, h @ wv       (16 heads of dim 64)
    att = softmax(causal(q k^T / 8)) v   -> [T, E]
    out = x + att @ w_proj + b_proj
    mlp = gelu(LN(out; ln2_g, ln2_b) @ w1 + b1) @ w2 + b2
    ret = out + mlp

Sharding: data-parallel over batch. B == 8 == n_cores, one batch element per
NeuronCore, no collectives. Each core runs the identical program on x[b].

Kernel layout strategy (per core):
  - LN1/LN2 computed in token layout [t, E] (free-dim reductions via bn_stats),
    then tiles are PE-transposed to e-partition layout hT/h2T [E, T], which is
    what every matmul needs (contraction dim on partitions).
  - All big matmuls use bf16 operands with fp32 PSUM accumulation (same PE
    rate as f32r, half the weight/activation SBUF+DMA bytes, FWL weight
    loads).  LN statistics, softmax normalization, residuals and the final
    output stay fp32.
  - Weights are pre-tiled on the host so every weight DMA is a contiguous
    2KB-per-partition transfer: wq/wk arrive as [pair, p, ec, n] and w1 as
    [fc, p, ec, n] stationary tiles; wv/w_proj/w2 stream as row-major
    [128, 512] moving tiles.  w1 is loaded into SBUF once and reused by both
    512-token passes.
  - Attention is computed transposed: scoresT[t_k, t_q] = k_h q_h^T so that the
    softmax denominator (sum over keys) can be produced by appending a ones
    column to v_h: attT_psum[65, t_q] = [v_h | 1]^T @ exp(scoresT).  Row 64 is
    the denominator; its reciprocal is partition-broadcast and multiplied in.
  - attnT head-pair tiles (partition = 2x64 head dims) feed the proj matmul as
    the stationary operand directly; proj output lands in token layout and is
    added to x in place (residual).  The MLP's first matmul produces uT [f, t]
    (transposed), so gelu's bias b1 is a per-partition ACT bias, and the second
    matmul consumes gelu(uT) as stationary, producing token-layout output that
    is added to the residual.
"""

import numpy as np

B, T, E = 8, 1024, 1024
NH, HD, FF = 16, 64, 4096
NPAIR = NH // 2          # 8 head pairs (2 heads per 128-partition tile)
EPS = 1e-5
NCORES = 8
TCH = T // 128           # 8 token chunks
ECH = E // 128           # 8 embedding chunks
FCH = FF // 128          # 32 mlp hidden chunks
TQW = 512                # moving-dim width for t
NTQ = T // TQW           # 2

_STAGES = {"ln": 1, "vqk": 2, "attn": 3, "proj": 4, "full": 5}

# fp8e4m3 + DoubleRow for the two MLP matmuls (~60% of the FLOPs at 2x PE
# rate).  w1/w2 are host-scaled into fp8's normal range; w1's scale is
# descaled inside the gelu activation, w2's at the residual add.
FP8_MLP = True
W1_SCALE = 16.0
W2_SCALE = 16.0


def _build_program(flags, stage="full", loop_n=1):
    """Build + compile the SPMD Bass program.

    flags: dict of bools controlling optional bias/gain application.
    stage: truncate the program after this phase and emit debug outputs.
    loop_n: if >1, wrap the whole body in an on-device For_i loop that
        recomputes the identical output loop_n times (used for timing).
    """
    import concourse.bass as bass
    import concourse.tile as tile
    from concourse import bacc, mybir
    from concourse.masks import make_identity, make_upper_triangular
    from contextlib import nullcontext

    sn = _STAGES[stage]
    f32 = mybir.dt.float32
    bf16 = mybir.dt.bfloat16
    AF = mybir.ActivationFunctionType

    nc = bacc.Bacc("TRN2", target_bir_lowering=False, debug=False,
                   num_devices=NCORES)

    x_d = nc.dram_tensor("x", [T, E], f32, kind="ExternalInput").ap()
    # wq/wk/w1 are host-pre-tiled into stationary-tile layout
    # [tile, p, chunk, n] so their DMA is fully contiguous per partition.
    wq_d = nc.dram_tensor("wq", [NPAIR, 128, ECH, 128], bf16,
                          kind="ExternalInput").ap()
    wk_d = nc.dram_tensor("wk", [NPAIR, 128, ECH, 128], bf16,
                          kind="ExternalInput").ap()
    wv_d = nc.dram_tensor("wv", [E, E], bf16, kind="ExternalInput").ap()
    wp_d = nc.dram_tensor("w_proj", [E, E], bf16, kind="ExternalInput").ap()
    if FP8_MLP:
        fp8 = mybir.dt.float8e4
        w1_d = nc.dram_tensor("w1", [FCH, 128, ECH, 128], fp8,
                              kind="ExternalInput").ap()
        w2_d = nc.dram_tensor("w2", [2, FF // 256, 128, 2, E // 2], fp8,
                              kind="ExternalInput").ap()
    else:
        w1_d = nc.dram_tensor("w1", [FCH, 128, ECH, 128], bf16,
                              kind="ExternalInput").ap()
        w2_d = nc.dram_tensor("w2", [FF, E], bf16,
                              kind="ExternalInput").ap()
    # host-pretiled to [p, fc] so the DMA is contiguous per partition
    b1_d = nc.dram_tensor("b1", [128, FCH], f32, kind="ExternalInput").ap()
    ln1g_d = ln1b_d = ln2g_d = ln2b_d = bp_d = b2_d = None
    if flags["ln1_gb"]:
        ln1g_d = nc.dram_tensor("ln1_g", [E], f32, kind="ExternalInput").ap()
        ln1b_d = nc.dram_tensor("ln1_b", [E], f32, kind="ExternalInput").ap()
    if flags["ln2_gb"]:
        ln2g_d = nc.dram_tensor("ln2_g", [E], f32, kind="ExternalInput").ap()
        ln2b_d = nc.dram_tensor("ln2_b", [E], f32, kind="ExternalInput").ap()
    if flags["b_proj"]:
        bp_d = nc.dram_tensor("b_proj", [E], f32, kind="ExternalInput").ap()
    if flags["b2"]:
        b2_d = nc.dram_tensor("b2", [E], f32, kind="ExternalInput").ap()
    out_d = nc.dram_tensor("out", [T, E], f32, kind="ExternalOutput").ap()

    dbg_outs = {}

    def dbg_tensor(name, shape):
        dbg_outs[name] = nc.dram_tensor(name, shape, f32,
                                        kind="ExternalOutput").ap()
        return dbg_outs[name]

    with tile.TileContext(nc) as tc:
        with (
            tc.For_i(0, loop_n, 1) if loop_n > 1 else nullcontext(),
            tc.tile_pool(name="resid", bufs=TCH) as p_resid,
            tc.tile_pool(name="ht", bufs=ECH) as p_ht,
            tc.tile_pool(name="htok", bufs=3) as p_htok,
            tc.tile_pool(name="small", bufs=6) as p_small,
            tc.tile_pool(name="singles", bufs=1) as p_single,
            tc.tile_pool(name="wsta", bufs=4) as p_wsta,
            tc.tile_pool(name="wmov", bufs=8) as p_wmov,
            tc.tile_pool(name="ps", bufs=8, space="PSUM") as p_ps,
        ):
            # ---- constants ----
            ident_f = p_single.tile([128, 128], f32, tag="identf",
                                    name="identf")
            make_identity(nc, ident_f[:])
            ident = p_single.tile([128, 128], bf16, tag="ident", name="ident")
            nc.vector.tensor_copy(ident[:], ident_f[:])
            # tri[k, q] = 1 if k <= q else 0 (upper triangular incl diagonal)
            tri_f = p_single.tile([128, 128], f32, tag="trif", name="trif")
            make_upper_triangular(nc, tri_f[:], val=1.0, diag=True)
            tri = p_single.tile([128, 128], bf16, tag="tri", name="tri")
            nc.vector.tensor_copy(tri[:], tri_f[:])
            ones16 = p_single.tile([128, NH, 1], bf16, tag="ones16",
                                   name="ones16")
            nc.vector.memset(ones16[:], 1.0)
            zer384 = p_single.tile([128, 384], bf16, tag="zer384",
                                   name="zer384")
            nc.vector.memset(zer384[:], 0.0)
            epst = p_single.tile([128, 1], f32, tag="epst", name="epst")
            nc.vector.memset(epst[:], EPS)
            b1c = p_single.tile([128, FCH], f32, tag="b1c", name="b1c")
            nc.sync.dma_start(b1c[:], b1_d)

            def bcast_row(dram_vec, tag, dt=f32):
                t_ = p_single.tile([128, E], f32, tag=tag, name=tag)
                src = bass.AP(tensor=dram_vec.tensor, offset=dram_vec.offset,
                              ap=[[0, 128]] + list(dram_vec.ap))
                nc.sync.dma_start(t_[:], src)
                if dt is f32:
                    return t_
                tb = p_single.tile([128, E], dt, tag=tag + "b", name=tag + "b")
                nc.vector.tensor_copy(tb[:], t_[:])
                return tb

            ln1g_b = bcast_row(ln1g_d, "ln1g", bf16) if flags["ln1_gb"] else None
            ln1b_b = bcast_row(ln1b_d, "ln1b", bf16) if flags["ln1_gb"] else None
            ln2g_b = bcast_row(ln2g_d, "ln2g", bf16) if flags["ln2_gb"] else None
            ln2b_b = bcast_row(ln2b_d, "ln2b", bf16) if flags["ln2_gb"] else None
            bp_b = bcast_row(bp_d, "bpb") if flags["b_proj"] else None
            b2_b = bcast_row(b2_d, "b2b") if flags["b2"] else None

            # ---- load x ----
            xt = []
            for tch in range(TCH):
                xt.append(p_resid.tile([128, E], f32, tag="resid",
                                       name="resid"))
                nc.sync.dma_start(xt[tch][:], x_d[128 * tch:128 * (tch + 1), :])

            # ---- layernorm in token layout + PE transpose to [E, T] ----
            # Two passes: normalize all token tiles first, then transpose
            # ec-major so each ht[ec] completes early and downstream matmuls
            # (which consume whole ht tiles) can start before LN finishes.
            def layer_norm_transposed(src_tiles, g_b, b_b, fmt="bf16"):
                if fmt == "bf16":
                    ht = [p_ht.tile([128, T], bf16, tag="ht", name="ht")
                          for _ in range(ECH)]
                else:   # fp8 e-chunk pairs for DoubleRow consumption
                    ht = [p_ht.tile([128, 2, T], mybir.dt.float8e4,
                                    tag="ht8", name="ht8")
                          for _ in range(ECH // 2)]
                hs = []
                for tch in range(TCH):
                    xti = src_tiles[tch]
                    st = p_small.tile([128, 2, 6], f32, tag="st", name="st")
                    nc.vector.bn_stats(st[:, 0, :], xti[:, 0:512])
                    nc.vector.bn_stats(st[:, 1, :], xti[:, 512:1024])
                    mv = p_small.tile([128, 2], f32, tag="mv", name="mv")
                    nc.vector.bn_aggr(mv[:], st[:])
                    sq = p_small.tile([128, 1], f32, tag="sq", name="sq")
                    nc.scalar.activation(sq[:], mv[:, 1:2], AF.Sqrt,
                                         bias=epst[:])
                    rsig = p_small.tile([128, 1], f32, tag="rsig", name="rsig")
                    nc.vector.reciprocal(rsig[:], sq[:])
                    h = p_htok.tile([128, E], bf16, tag="htok", name="htok",
                                    bufs=TCH)
                    nc.vector.tensor_scalar(h[:], xti[:], mv[:, 0:1],
                                            rsig[:], mybir.AluOpType.subtract,
                                            mybir.AluOpType.mult)
                    if g_b is not None:
                        nc.vector.tensor_mul(h[:], h[:], g_b[:])
                        nc.vector.tensor_add(h[:], h[:], b_b[:])
                    hs.append(h)
                for ec in range(ECH):
                    for tch in range(TCH):
                        pst = p_ps.tile([128, 128], bf16, tag="ps",
                                        name="ps")
                        nc.tensor.transpose(pst[:],
                                            hs[tch][:, 128 * ec:128 * (ec + 1)],
                                            ident[:])
                        if fmt == "bf16":
                            dst = ht[ec][:, 128 * tch:128 * (tch + 1)]
                        else:
                            dst = ht[ec // 2][:, ec % 2,
                                              128 * tch:128 * (tch + 1)]
                        nc.vector.tensor_copy(dst, pst[:])
                return ht

            ht = layer_norm_transposed(xt, ln1g_b, ln1b_b)

            def dump_f32(dst, src_bf16):
                stg = p_htok.tile([128, src_bf16.shape[-1]], f32, tag="dump",
                                  name="dump")
                nc.vector.tensor_copy(stg[:], src_bf16)
                nc.sync.dma_start(dst, stg[:])

            if sn == 1:
                o = dbg_tensor("dbg_ht", [E, T])
                for ec in range(ECH):
                    dump_f32(o[128 * ec:128 * (ec + 1), :], ht[ec][:])

            if sn >= 2:
                # attention-phase pools; closed before the MLP phase
                att_cms = [
                    tc.tile_pool(name="qk", bufs=3),
                    tc.tile_pool(name="vpool", bufs=TCH),
                    tc.tile_pool(name="esc", bufs=6),
                    tc.tile_pool(name="attn", bufs=NPAIR),
                    tc.tile_pool(name="norm", bufs=3),
                ]
                p_qk, p_v, p_esc, p_attn, p_norm = (
                    cm.__enter__() for cm in att_cms)

                # ---- V = h @ wv -> token layout [t, head, 65] + ones col ----
                vt = []
                for tch in range(TCH):
                    v = p_v.tile([128, NH, HD + 1], bf16, tag="v", name="v")
                    nc.vector.tensor_copy(v[:, :, HD:HD + 1], ones16[:])
                    vt.append(v)
                for half in range(2):
                    esl = slice(512 * half, 512 * (half + 1))
                    ys = [p_ps.tile([128, 512], f32, tag="ps", name="ps")
                          for _ in range(TCH)]
                    for ec in range(ECH):
                        wv_t = p_wmov.tile([128, 512], bf16, tag="wmov",
                                           name="wmov")
                        nc.sync.dma_start(wv_t[:],
                                          wv_d[128 * ec:128 * (ec + 1), esl])
                        for tch in range(TCH):
                            nc.tensor.matmul(
                                ys[tch][:],
                                ht[ec][:, 128 * tch:128 * (tch + 1)],
                                wv_t[:], start=(ec == 0),
                                stop=(ec == ECH - 1))
                    for tch in range(TCH):
                        nc.vector.tensor_copy(
                            vt[tch][:, 8 * half:8 * (half + 1), 0:HD],
                            ys[tch][:].rearrange("p (h d) -> p h d", d=HD))

                # ---- per head pair: qT/kT, scores, softmax, att ----
                attn_t = []
                for pair in range(NPAIR if sn >= 3 else 1):
                    wq_t = p_wsta.tile([128, ECH, 128], bf16, tag="wsta",
                                       name="wsta")
                    nc.sync.dma_start(wq_t[:], wq_d[pair])
                    wk_t = p_wsta.tile([128, ECH, 128], bf16, tag="wsta",
                                       name="wsta")
                    nc.sync.dma_start(wk_t[:], wk_d[pair])
                    qT = p_qk.tile([128, T], bf16, tag="qk", name="qk")
                    kT = p_qk.tile([128, T], bf16, tag="qk", name="qk")
                    for (w_t, dst) in ((wq_t, qT), (wk_t, kT)):
                        for th in range(NTQ):
                            tsl = slice(TQW * th, TQW * (th + 1))
                            ps = p_ps.tile([128, 512], f32, tag="ps",
                                           name="ps")
                            for ec in range(ECH):
                                nc.tensor.matmul(
                                    ps[:], w_t[:, ec, :], ht[ec][:, tsl],
                                    start=(ec == 0), stop=(ec == ECH - 1))
                            nc.vector.tensor_copy(dst[:, tsl], ps[:])

                    if sn == 2 and pair == 0:
                        oq = dbg_tensor("dbg_qT", [128, T])
                        dump_f32(oq[:, :], qT[:])
                        ok_ = dbg_tensor("dbg_kT", [128, T])
                        dump_f32(ok_[:, :], kT[:])
                        break

                    att_pair = p_attn.tile([128, T], bf16, tag="attn",
                                           name="attn")
                    attn_t.append(att_pair)
                    # both heads of the pair interleaved: the two score
                    # matmuls (K=64, stationary base_partition 0 / 64 ->
                    # row-groups (0,0)/(64,0)) are emitted back-to-back so
                    # the PE runs them concurrently in different row groups.
                    for bq in range(NTQ):
                        qsl = slice(TQW * bq, TQW * (bq + 1))
                        nbk = min(TCH, 4 * bq + 4)
                        ps_a = [p_ps.tile([128, 512], f32, tag="ps",
                                          name="ps") for _ in range(2)]
                        for bk in range(nbk):
                            d = bk - 4 * bq
                            ets = []
                            pss = []
                            for hp in range(2):
                                rows = slice(HD * hp, HD * (hp + 1))
                                ps_s = p_ps.tile([128, 512], f32, tag="ps",
                                                 name="ps")
                                nc.tensor.matmul(
                                    ps_s[:],
                                    kT[rows, 128 * bk:128 * (bk + 1)],
                                    qT[rows, qsl], start=True, stop=True)
                                pss.append(ps_s)
                            for hp in range(2):
                                ps_s = pss[hp]
                                et = p_esc.tile([128, 512], bf16, tag="esc",
                                                name="esc")
                                if d <= 0:
                                    nc.scalar.activation(et[:], ps_s[:],
                                                         AF.Exp, scale=0.125)
                                else:
                                    nc.vector.tensor_copy(
                                        et[:, 0:128 * d],
                                        zer384[:, 0:128 * d])
                                    nc.scalar.activation(
                                        et[:, 128 * d:512],
                                        ps_s[:, 128 * d:512],
                                        AF.Exp, scale=0.125)
                                if d >= 0:
                                    dsl = slice(128 * d, 128 * (d + 1))
                                    nc.vector.tensor_mul(et[:, dsl],
                                                         et[:, dsl], tri[:])
                                ets.append(et)
                            for hp in range(2):
                                nc.tensor.matmul(
                                    ps_a[hp][0:HD + 1, :],
                                    vt[bk][:, 2 * pair + hp, :],
                                    ets[hp][:], start=(bk == 0),
                                    stop=(bk == nbk - 1))
                        # normalize by the denominator (row HD of ps_a)
                        for hp in range(2):
                            rcp = p_norm.tile([HD + 1, 512], f32, tag="rcp",
                                              name="rcp")
                            nc.vector.reciprocal(rcp[HD:HD + 1, :],
                                                 ps_a[hp][HD:HD + 1, :])
                            bct = p_norm.tile([HD, 512], f32, tag="bct",
                                              name="bct")
                            rsl = rcp[HD:HD + 1, :]
                            rap = list(rsl.ap)
                            rbc = bass.AP(tensor=rsl.tensor, offset=rsl.offset,
                                          ap=[rap[0], [0, HD], rap[1]])
                            nc.gpsimd.dma_start(out=bct[:], in_=rbc)
                            if hp == 0:
                                nc.vector.tensor_mul(att_pair[0:HD, qsl],
                                                     ps_a[hp][0:HD, :],
                                                     bct[:])
                            else:
                                sc = p_norm.tile([HD, 512], bf16,
                                                 tag="oddsc", name="oddsc")
                                nc.vector.tensor_mul(sc[:], ps_a[hp][0:HD, :],
                                                     bct[:])
                                nc.sync.dma_start(att_pair[HD:128, qsl],
                                                  sc[:])

                if sn == 2:
                    o2 = dbg_tensor("dbg_v", [T, NH * (HD + 1)])
                    for tch in range(TCH):
                        dump_f32(o2[128 * tch:128 * (tch + 1), :],
                                 vt[tch][:].rearrange("p h d -> p (h d)"))
                if sn == 3:
                    o = dbg_tensor("dbg_attnT", [E, T])
                    for pr in range(NPAIR):
                        dump_f32(o[128 * pr:128 * (pr + 1), :],
                                 attn_t[pr][:])

                # ---- out = x + attnT^T @ w_proj (+ b_proj), in-place xt ----
                if sn >= 4:
                    for eo in range(2):
                        esl = slice(512 * eo, 512 * (eo + 1))
                        ys = [p_ps.tile([128, 512], f32, tag="ps", name="ps")
                              for _ in range(TCH)]
                        for pair in range(NPAIR):
                            wp_t = p_wmov.tile([128, 512], bf16, tag="wmov",
                                               name="wmov")
                            nc.sync.dma_start(
                                wp_t[:],
                                wp_d[128 * pair:128 * (pair + 1), esl])
                            for tch in range(TCH):
                                nc.tensor.matmul(
                                    ys[tch][:],
                                    attn_t[pair][:, 128 * tch:128 * (tch + 1)],
                                    wp_t[:], start=(pair == 0),
                                    stop=(pair == NPAIR - 1))
                        for tch in range(TCH):
                            nc.vector.tensor_add(xt[tch][:, esl], ys[tch][:],
                                                 xt[tch][:, esl])
                            if flags["b_proj"]:
                                nc.vector.tensor_add(xt[tch][:, esl],
                                                     xt[tch][:, esl],
                                                     bp_b[:, esl])

                for cm in reversed(att_cms):
                    cm.__exit__(None, None, None)

            if sn == 4:
                o = dbg_tensor("dbg_out1", [T, E])
                for tch in range(TCH):
                    nc.sync.dma_start(o[128 * tch:128 * (tch + 1), :],
                                      xt[tch][:])

            if sn >= 5:
                # ---- LN2 + transpose ----
                h2t = layer_norm_transposed(
                    xt, ln2g_b, ln2b_b, fmt=("fp8" if FP8_MLP else "bf16"))

                # ---- MLP: uT = w1^T h2T; g = gelu(uT+b1); y = g^T w2 ----
                # w1 is DMA'd into SBUF once and reused by both 512-token
                # passes; w2 streams once per pass.
                DR = mybir.MatmulPerfMode.DoubleRow if FP8_MLP else None
                with (tc.tile_pool(name="w1sta", bufs=FCH) as p_w1,
                      tc.tile_pool(name="gt", bufs=FCH) as p_gt):
                    w1_t = []
                    for fc in range(FCH):
                        w = p_w1.tile([128, ECH, 128],
                                      mybir.dt.float8e4 if FP8_MLP else bf16,
                                      tag="w1", name="w1")
                        nc.sync.dma_start(w[:], w1_d[fc])
                        w1_t.append(w)
                    for th in range(NTQ):
                        tsl = slice(TQW * th, TQW * (th + 1))
                        gt = []
                        if FP8_MLP:
                            gt = [p_gt.tile([128, 2, 512], mybir.dt.float8e4,
                                            tag="gt", name="gt")
                                  for _ in range(FCH // 2)]
                        for fc in range(FCH):
                            ps = p_ps.tile([128, 512], f32, tag="ps",
                                           name="ps")
                            if FP8_MLP:
                                for j in range(ECH // 2):
                                    nc.tensor.matmul(
                                        ps[:], w1_t[fc][:, 2 * j:2 * j + 2, :],
                                        h2t[j][:, :, tsl],
                                        start=(j == 0),
                                        stop=(j == ECH // 2 - 1),
                                        perf_mode=DR)
                                nc.scalar.activation(
                                    gt[fc // 2][:, fc % 2, :], ps[:], AF.Gelu,
                                    bias=b1c[:, fc:fc + 1],
                                    scale=1.0 / W1_SCALE)
                            else:
                                for ec in range(ECH):
                                    nc.tensor.matmul(
                                        ps[:], w1_t[fc][:, ec, :],
                                        h2t[ec][:, tsl],
                                        start=(ec == 0), stop=(ec == ECH - 1))
                                g = p_gt.tile([128, 512], bf16, tag="gt",
                                              name="gt")
                                nc.scalar.activation(g[:], ps[:], AF.Gelu,
                                                     bias=b1c[:, fc:fc + 1])
                                gt.append(g)
                        for eo in range(2):
                            esl = slice(512 * eo, 512 * (eo + 1))
                            ys = [p_ps.tile([128, 512], f32, tag="ps",
                                            name="ps") for _ in range(4)]
                            if FP8_MLP:
                                for j2 in range(FF // 256):
                                    w2_t = p_wmov.tile(
                                        [128, 2, 512], mybir.dt.float8e4,
                                        tag="wmov8", name="wmov8")
                                    nc.sync.dma_start(w2_t[:],
                                                      w2_d[eo, j2])
                                    for tl in range(4):
                                        nc.tensor.matmul(
                                            ys[tl][:],
                                            gt[j2][:, :,
                                                   128 * tl:128 * (tl + 1)],
                                            w2_t[:], start=(j2 == 0),
                                            stop=(j2 == FF // 256 - 1),
                                            perf_mode=DR)
                            else:
                                for fc in range(FCH):
                                    w2_t = p_wmov.tile([128, 512], bf16,
                                                       tag="wmov",
                                                       name="wmov")
                                    nc.sync.dma_start(
                                        w2_t[:],
                                        w2_d[128 * fc:128 * (fc + 1), esl])
                                    for tl in range(4):
                                        nc.tensor.matmul(
                                            ys[tl][:],
                                            gt[fc][:,
                                                   128 * tl:128 * (tl + 1)],
                                            w2_t[:], start=(fc == 0),
                                            stop=(fc == FCH - 1))
                            for tl in range(4):
                                tch = 4 * th + tl
                                if FP8_MLP:
                                    ydsc = p_htok.tile([128, 512], f32,
                                                       tag="ydsc",
                                                       name="ydsc", bufs=4)
                                    nc.scalar.activation(
                                        ydsc[:], ys[tl][:], AF.Copy,
                                        scale=1.0 / W2_SCALE)
                                    nc.vector.tensor_add(xt[tch][:, esl],
                                                         ydsc[:],
                                                         xt[tch][:, esl])
                                else:
                                    nc.vector.tensor_add(xt[tch][:, esl],
                                                         ys[tl][:],
                                                         xt[tch][:, esl])
                                if flags["b2"]:
                                    nc.vector.tensor_add(xt[tch][:, esl],
                                                         xt[tch][:, esl],
                                                         b2_b[:, esl])
                        for tl in range(4):
                            tch = 4 * th + tl
                            nc.sync.dma_start(
                                out_d[128 * tch:128 * (tch + 1), :],
                                xt[tch][:])

    nc.compile()
    return nc, dbg_outs


_CACHE = {}


def _get_program(flags_key, stage="full", loop_n=1):
    key = (flags_key, stage, loop_n)
    if key not in _CACHE:
        flags = dict(zip(("ln1_gb", "ln2_gb", "b_proj", "b2"), flags_key))
        _CACHE[key] = _build_program(flags, stage, loop_n=loop_n)
    return _CACHE[key]


def _flags_for(inputs):
    return {
        "ln1_gb": not (np.all(np.asarray(inputs["ln1_g"]) == 1.0)
                       and np.all(np.asarray(inputs["ln1_b"]) == 0.0)),
        "ln2_gb": not (np.all(np.asarray(inputs["ln2_g"]) == 1.0)
                       and np.all(np.asarray(inputs["ln2_b"]) == 0.0)),
        "b_proj": not np.all(np.asarray(inputs["b_proj"]) == 0.0),
        "b2": not np.all(np.asarray(inputs["b2"]) == 0.0),
    }


def _make_in_maps(inputs, flags, cores):
    from concourse import mybir

    bf16 = mybir.dt.np(mybir.dt.bfloat16)
    common = {}
    # stationary-tile pre-tiling: [e_in, n_out] -> [tile, p, chunk, n] with
    # e_in = chunk*128 + p and n_out = tile*128 + n
    for name, ntile in (("wq", NPAIR), ("wk", NPAIR), ("w1", FCH)):
        w = np.asarray(inputs[name], np.float32)
        w = w.reshape(ECH, 128, ntile, 128).transpose(2, 1, 0, 3)
        common[name] = np.ascontiguousarray(w).astype(bf16)
    for name in ("wv", "w_proj", "w2"):
        common[name] = np.ascontiguousarray(
            np.asarray(inputs[name], np.float32)).astype(bf16)
    if FP8_MLP:
        fp8 = mybir.dt.np(mybir.dt.float8e4)
        w1 = np.asarray(inputs["w1"], np.float32)
        w1 = w1.reshape(ECH, 128, FCH, 128).transpose(2, 1, 0, 3) * W1_SCALE
        common["w1"] = np.ascontiguousarray(w1).astype(fp8)
        # w2 [FF, E] -> [eo, j2, p, sub, n] with f = j2*256 + sub*128 + p
        # and e = eo*512 + n, so each [128, 2, 512] moving tile is a fully
        # contiguous per-partition DMA.
        w2 = np.asarray(inputs["w2"], np.float32)
        w2 = w2.reshape(FF // 256, 2, 128, 2, E // 2)
        w2 = w2.transpose(3, 0, 2, 1, 4) * W2_SCALE
        common["w2"] = np.ascontiguousarray(w2).astype(fp8)
    # b1 [FF] -> [p, fc] with f = fc*128 + p
    common["b1"] = np.ascontiguousarray(
        np.asarray(inputs["b1"], np.float32).reshape(FCH, 128).T)
    for name, flag in (("ln1_g", "ln1_gb"), ("ln1_b", "ln1_gb"),
                       ("ln2_g", "ln2_gb"), ("ln2_b", "ln2_gb"),
                       ("b_proj", "b_proj"), ("b2", "b2")):
        if flags[flag]:
            common[name] = np.ascontiguousarray(inputs[name], np.float32)
    x = np.ascontiguousarray(inputs["x"], np.float32)
    return [{"x": x[c], **common} for c in cores]


def _run(inputs, stage="full", cores=None):
    from concourse.bass_utils import run_bass_kernel_spmd

    if cores is None:
        cores = list(range(NCORES))
    flags = _flags_for(inputs)
    flags_key = tuple(flags[k] for k in ("ln1_gb", "ln2_gb", "b_proj", "b2"))
    nc, dbg = _get_program(flags_key, stage)
    in_maps = _make_in_maps(inputs, flags, cores)
    res = run_bass_kernel_spmd(nc, in_maps, cores)
    return res, dbg


_FN_CACHE = {}


def kernel(**inputs) -> np.ndarray:
    import jax

    flags = _flags_for(inputs)
    flags_key = tuple(flags[k] for k in ("ln1_gb", "ln2_gb", "b_proj", "b2"))
    nc, _ = _get_program(flags_key, "full")
    if id(nc) not in _FN_CACHE:
        _FN_CACHE[id(nc)] = _make_jit_fn(nc)
    fn, mesh, in_names, out_names, out_avals = _FN_CACHE[id(nc)]
    in_maps = _make_in_maps(inputs, flags, list(range(NCORES)))
    dev_args = _device_args(in_maps, in_names, out_avals, mesh)
    out = fn(*dev_args)
    jax.block_until_ready(out)
    oi = out_names.index("out")
    return np.asarray(out[oi]).reshape(NCORES, T, E)


def _make_jit_fn(nc):
    """Build a jitted SPMD executor for a compiled Bass program.

    Returns (fn, mesh, in_names, out_names, out_avals)."""
    import jax
    from jax.experimental.shard_map import shard_map
    from jax.sharding import Mesh, PartitionSpec
    from concourse import mybir
    from concourse.bass2jax import (_bass_exec_p, install_neuronx_cc_hook,
                                    partition_id_tensor)

    install_neuronx_cc_hook()
    partition_name = (nc.partition_id_tensor.name
                      if nc.partition_id_tensor else None)
    in_names, out_names, out_avals = [], [], []
    for alloc in nc.m.functions[0].allocations:
        if not isinstance(alloc, mybir.MemoryLocationSet):
            continue
        name = alloc.memorylocations[0].name
        if alloc.kind == "ExternalInput":
            if name != partition_name:
                in_names.append(name)
        elif alloc.kind == "ExternalOutput":
            out_names.append(name)
            shape = tuple(alloc.tensor_shape)
            dtype = mybir.dt.np(alloc.dtype)
            out_avals.append(jax.core.ShapedArray(shape, dtype))
    n_params = len(in_names)
    all_names = in_names + out_names
    if partition_name is not None:
        all_names = all_names + [partition_name]

    def _body(*args):
        operands = list(args)
        if partition_name is not None:
            operands.append(partition_id_tensor())
        outs = _bass_exec_p.bind(
            *operands,
            out_avals=tuple(out_avals),
            in_names=tuple(all_names),
            out_names=tuple(out_names),
            lowering_input_output_aliases=(),
            sim_require_finite=True,
            sim_require_nnan=True,
            nc=nc,
        )
        return tuple(outs)

    devices = jax.devices()[:NCORES]
    mesh = Mesh(np.asarray(devices), ("core",))
    n_outs = len(out_names)
    in_specs = (PartitionSpec("core"),) * (n_params + n_outs)
    out_specs = (PartitionSpec("core"),) * n_outs
    fn = jax.jit(shard_map(_body, mesh=mesh, in_specs=in_specs,
                           out_specs=out_specs, check_rep=False),
                 keep_unused=True)
    return fn, mesh, in_names, out_names, out_avals


def _device_args(in_maps, in_names, out_avals, mesh):
    """device_put the per-core inputs pre-sharded across the mesh (one hop,
    no per-call resharding) plus zeroed output buffers."""
    import jax
    from jax.sharding import NamedSharding, PartitionSpec

    sharding = NamedSharding(mesh, PartitionSpec("core"))
    concat_in = [
        np.concatenate([np.asarray(in_maps[c][nm]) for c in range(NCORES)],
                       axis=0)
        for nm in in_names
    ]
    concat_zeros = [
        np.zeros((NCORES * a.shape[0], *a.shape[1:]), a.dtype)
        for a in out_avals
    ]
    return [jax.device_put(a, sharding) for a in concat_in + concat_zeros]


def _timed_run(inputs, iters=10, stage="full"):
    """Measure per-application device time of the kernel.

    Two variants of the same program are run: one computing the block once,
    and one recomputing it LOOP_N times in an on-device For_i loop (every
    iteration reads the same DRAM inputs and rewrites the same output, so
    the result is identical).  Inputs are device-resident and pre-sharded;
    the marginal time (t_loop - t_single) / (LOOP_N - 1), min over `iters`
    wall-clock reps of each, is the per-iteration hardware execution time
    with the fixed dispatch/tunnel overhead cancelled out.

    Returns (out [B,T,E], seconds_per_iter).
    """
    import time
    import jax

    LOOP_N = 41
    flags = _flags_for(inputs)
    flags_key = tuple(flags[k] for k in ("ln1_gb", "ln2_gb", "b_proj", "b2"))
    nc1, _ = _get_program(flags_key, stage)
    ncN, _ = _get_program(flags_key, stage, loop_n=LOOP_N)
    in_maps = _make_in_maps(inputs, flags, list(range(NCORES)))

    fn1, mesh, in_names, out_names, out_avals = _make_jit_fn(nc1)
    fnN, _, _, _, _ = _make_jit_fn(ncN)
    dev_args = _device_args(in_maps, in_names, out_avals, mesh)

    out1 = fn1(*dev_args)
    jax.block_until_ready(out1)     # warm-up (compile + first run)
    outN = fnN(*dev_args)
    jax.block_until_ready(outN)
    # Interleaved single/loop calls; congestion spikes in the tunnel or on
    # the device are strictly additive, so min-over-reps of each estimates
    # the uncongested time and their difference the marginal per-iteration
    # hardware time.
    t_single, t_loop = [], []
    for _ in range(max(iters, 20)):
        t0 = time.perf_counter()
        out1 = fn1(*dev_args)
        jax.block_until_ready(out1)
        t1 = time.perf_counter()
        outN = fnN(*dev_args)
        jax.block_until_ready(outN)
        t2 = time.perf_counter()
        t_single.append(t1 - t0)
        t_loop.append(t2 - t1)
    per_iter = max(0.0, min(t_loop) - min(t_single)) / (LOOP_N - 1)
    oi = out_names.index("out")
    res = np.asarray(outN[oi]).reshape(NCORES, T, E)
    return res, per_iter



# revision 50
# speedup vs baseline: 1.0206x; 1.0206x over previous
"""Trainium2 Bass kernel for a dense transformer block (pre-LN, causal MHA + GELU MLP).

Reference computation (per batch element b, all fp32):
    h   = LN(x; ln1_g, ln1_b)
    q,k,v = h @ wq, h @ wk, h @ wv       (16 heads of dim 64)
    att = softmax(causal(q k^T / 8)) v   -> [T, E]
    out = x + att @ w_proj + b_proj
    mlp = gelu(LN(out; ln2_g, ln2_b) @ w1 + b1) @ w2 + b2
    ret = out + mlp

Sharding: data-parallel over batch. B == 8 == n_cores, one batch element per
NeuronCore, no collectives. Each core runs the identical program on x[b].

Kernel layout strategy (per core):
  - LN1/LN2 computed in token layout [t, E] (free-dim reductions via bn_stats),
    then tiles are PE-transposed to e-partition layout hT/h2T [E, T], which is
    what every matmul needs (contraction dim on partitions).
  - All big matmuls use bf16 operands with fp32 PSUM accumulation (same PE
    rate as f32r, half the weight/activation SBUF+DMA bytes, FWL weight
    loads).  LN statistics, softmax normalization, residuals and the final
    output stay fp32.
  - Weights are pre-tiled on the host so every weight DMA is a contiguous
    2KB-per-partition transfer: wq/wk arrive as [pair, p, ec, n] and w1 as
    [fc, p, ec, n] stationary tiles; wv/w_proj/w2 stream as row-major
    [128, 512] moving tiles.  w1 is loaded into SBUF once and reused by both
    512-token passes.
  - Attention is computed transposed: scoresT[t_k, t_q] = k_h q_h^T so that the
    softmax denominator (sum over keys) can be produced by appending a ones
    column to v_h: attT_psum[65, t_q] = [v_h | 1]^T @ exp(scoresT).  Row 64 is
    the denominator; its reciprocal is partition-broadcast and multiplied in.
  - attnT head-pair tiles (partition = 2x64 head dims) feed the proj matmul as
    the stationary operand directly; proj output lands in token layout and is
    added to x in place (residual).  The MLP's first matmul produces uT [f, t]
    (transposed), so gelu's bias b1 is a per-partition ACT bias, and the second
    matmul consumes gelu(uT) as stationary, producing token-layout output that
    is added to the residual.
"""

import numpy as np

B, T, E = 8, 1024, 1024
NH, HD, FF = 16, 64, 4096
NPAIR = NH // 2          # 8 head pairs (2 heads per 128-partition tile)
EPS = 1e-5
NCORES = 8
TCH = T // 128           # 8 token chunks
ECH = E // 128           # 8 embedding chunks
FCH = FF // 128          # 32 mlp hidden chunks
TQW = 512                # moving-dim width for t
NTQ = T // TQW           # 2

_STAGES = {"ln": 1, "vqk": 2, "attn": 3, "proj": 4, "full": 5}

# fp8e4m3 + DoubleRow for the two MLP matmuls (~60% of the FLOPs at 2x PE
# rate).  w1/w2 are host-scaled into fp8's normal range; w1's scale is
# descaled inside the gelu activation, w2's at the residual add.
FP8_MLP = True
W1_SCALE = 16.0
W2_SCALE = 16.0


def _build_program(flags, stage="full", loop_n=1):
    """Build + compile the SPMD Bass program.

    flags: dict of bools controlling optional bias/gain application.
    stage: truncate the program after this phase and emit debug outputs.
    loop_n: if >1, wrap the whole body in an on-device For_i loop that
        recomputes the identical output loop_n times (used for timing).
    """
    import concourse.bass as bass
    import concourse.tile as tile
    from concourse import bacc, mybir
    from concourse.masks import make_identity, make_upper_triangular
    from contextlib import nullcontext

    sn = _STAGES[stage]
    f32 = mybir.dt.float32
    bf16 = mybir.dt.bfloat16
    AF = mybir.ActivationFunctionType

    nc = bacc.Bacc("TRN2", target_bir_lowering=False, debug=False,
                   num_devices=NCORES)

    x_d = nc.dram_tensor("x", [T, E], f32, kind="ExternalInput").ap()
    # wq/wk/w1 are host-pre-tiled into stationary-tile layout
    # [tile, p, chunk, n] so their DMA is fully contiguous per partition.
    wq_d = nc.dram_tensor("wq", [NPAIR, 128, ECH, 128], bf16,
                          kind="ExternalInput").ap()
    wk_d = nc.dram_tensor("wk", [NPAIR, 128, ECH, 128], bf16,
                          kind="ExternalInput").ap()
    wv_d = nc.dram_tensor("wv", [E, E], bf16, kind="ExternalInput").ap()
    wp_d = nc.dram_tensor("w_proj", [E, E], bf16, kind="ExternalInput").ap()
    if FP8_MLP:
        fp8 = mybir.dt.float8e4
        w1_d = nc.dram_tensor("w1", [FCH, 128, ECH, 128], fp8,
                              kind="ExternalInput").ap()
        w2_d = nc.dram_tensor("w2", [2, FF // 256, 128, 2, E // 2], fp8,
                              kind="ExternalInput").ap()
    else:
        w1_d = nc.dram_tensor("w1", [FCH, 128, ECH, 128], bf16,
                              kind="ExternalInput").ap()
        w2_d = nc.dram_tensor("w2", [FF, E], bf16,
                              kind="ExternalInput").ap()
    # host-pretiled to [p, fc] so the DMA is contiguous per partition
    b1_d = nc.dram_tensor("b1", [128, FCH], f32, kind="ExternalInput").ap()
    ln1g_d = ln1b_d = ln2g_d = ln2b_d = bp_d = b2_d = None
    if flags["ln1_gb"]:
        ln1g_d = nc.dram_tensor("ln1_g", [E], f32, kind="ExternalInput").ap()
        ln1b_d = nc.dram_tensor("ln1_b", [E], f32, kind="ExternalInput").ap()
    if flags["ln2_gb"]:
        ln2g_d = nc.dram_tensor("ln2_g", [E], f32, kind="ExternalInput").ap()
        ln2b_d = nc.dram_tensor("ln2_b", [E], f32, kind="ExternalInput").ap()
    if flags["b_proj"]:
        bp_d = nc.dram_tensor("b_proj", [E], f32, kind="ExternalInput").ap()
    if flags["b2"]:
        b2_d = nc.dram_tensor("b2", [E], f32, kind="ExternalInput").ap()
    out_d = nc.dram_tensor("out", [T, E], f32, kind="ExternalOutput").ap()

    dbg_outs = {}

    def dbg_tensor(name, shape):
        dbg_outs[name] = nc.dram_tensor(name, shape, f32,
                                        kind="ExternalOutput").ap()
        return dbg_outs[name]

    with tile.TileContext(nc) as tc:
        with (
            tc.For_i(0, loop_n, 1) if loop_n > 1 else nullcontext(),
            tc.tile_pool(name="resid", bufs=TCH) as p_resid,
            tc.tile_pool(name="ht", bufs=ECH) as p_ht,
            tc.tile_pool(name="htok", bufs=3) as p_htok,
            tc.tile_pool(name="small", bufs=6) as p_small,
            tc.tile_pool(name="singles", bufs=1) as p_single,
            tc.tile_pool(name="wsta", bufs=4) as p_wsta,
            tc.tile_pool(name="wmov", bufs=8) as p_wmov,
            tc.tile_pool(name="ps", bufs=8, space="PSUM") as p_ps,
        ):
            # ---- constants ----
            ident_f = p_single.tile([128, 128], f32, tag="identf",
                                    name="identf")
            make_identity(nc, ident_f[:])
            ident = p_single.tile([128, 128], bf16, tag="ident", name="ident")
            nc.vector.tensor_copy(ident[:], ident_f[:])
            # tri[k, q] = 1 if k <= q else 0 (upper triangular incl diagonal)
            tri_f = p_single.tile([128, 128], f32, tag="trif", name="trif")
            make_upper_triangular(nc, tri_f[:], val=1.0, diag=True)
            tri = p_single.tile([128, 128], bf16, tag="tri", name="tri")
            nc.vector.tensor_copy(tri[:], tri_f[:])
            ones16 = p_single.tile([128, NH, 1], bf16, tag="ones16",
                                   name="ones16")
            nc.vector.memset(ones16[:], 1.0)
            zer384 = p_single.tile([128, 384], bf16, tag="zer384",
                                   name="zer384")
            nc.vector.memset(zer384[:], 0.0)
            epst = p_single.tile([128, 1], f32, tag="epst", name="epst")
            nc.vector.memset(epst[:], EPS)
            b1c = p_single.tile([128, FCH], f32, tag="b1c", name="b1c")
            nc.sync.dma_start(b1c[:], b1_d)

            def bcast_row(dram_vec, tag, dt=f32):
                t_ = p_single.tile([128, E], f32, tag=tag, name=tag)
                src = bass.AP(tensor=dram_vec.tensor, offset=dram_vec.offset,
                              ap=[[0, 128]] + list(dram_vec.ap))
                nc.sync.dma_start(t_[:], src)
                if dt is f32:
                    return t_
                tb = p_single.tile([128, E], dt, tag=tag + "b", name=tag + "b")
                nc.vector.tensor_copy(tb[:], t_[:])
                return tb

            ln1g_b = bcast_row(ln1g_d, "ln1g", bf16) if flags["ln1_gb"] else None
            ln1b_b = bcast_row(ln1b_d, "ln1b", bf16) if flags["ln1_gb"] else None
            ln2g_b = bcast_row(ln2g_d, "ln2g", bf16) if flags["ln2_gb"] else None
            ln2b_b = bcast_row(ln2b_d, "ln2b", bf16) if flags["ln2_gb"] else None
            bp_b = bcast_row(bp_d, "bpb") if flags["b_proj"] else None
            b2_b = bcast_row(b2_d, "b2b") if flags["b2"] else None

            # ---- load x ----
            xt = []
            for tch in range(TCH):
                xt.append(p_resid.tile([128, E], f32, tag="resid",
                                       name="resid"))
                nc.sync.dma_start(xt[tch][:], x_d[128 * tch:128 * (tch + 1), :])

            # ---- layernorm in token layout + PE transpose to [E, T] ----
            # Two passes: normalize all token tiles first, then transpose
            # ec-major so each ht[ec] completes early and downstream matmuls
            # (which consume whole ht tiles) can start before LN finishes.
            def layer_norm_transposed(src_tiles, g_b, b_b, fmt="bf16"):
                if fmt == "bf16":
                    ht = [p_ht.tile([128, T], bf16, tag="ht", name="ht")
                          for _ in range(ECH)]
                else:   # fp8 e-chunk pairs for DoubleRow consumption
                    ht = [p_ht.tile([128, 2, T], mybir.dt.float8e4,
                                    tag="ht8", name="ht8")
                          for _ in range(ECH // 2)]
                hs = []
                for tch in range(TCH):
                    xti = src_tiles[tch]
                    st = p_small.tile([128, 2, 6], f32, tag="st", name="st")
                    nc.vector.bn_stats(st[:, 0, :], xti[:, 0:512])
                    nc.vector.bn_stats(st[:, 1, :], xti[:, 512:1024])
                    mv = p_small.tile([128, 2], f32, tag="mv", name="mv")
                    nc.vector.bn_aggr(mv[:], st[:])
                    sq = p_small.tile([128, 1], f32, tag="sq", name="sq")
                    nc.scalar.activation(sq[:], mv[:, 1:2], AF.Sqrt,
                                         bias=epst[:])
                    rsig = p_small.tile([128, 1], f32, tag="rsig", name="rsig")
                    nc.vector.reciprocal(rsig[:], sq[:])
                    h = p_htok.tile([128, E], bf16, tag="htok", name="htok",
                                    bufs=TCH)
                    nc.vector.tensor_scalar(h[:], xti[:], mv[:, 0:1],
                                            rsig[:], mybir.AluOpType.subtract,
                                            mybir.AluOpType.mult)
                    if g_b is not None:
                        nc.vector.tensor_mul(h[:], h[:], g_b[:])
                        nc.vector.tensor_add(h[:], h[:], b_b[:])
                    hs.append(h)
                for ec in range(ECH):
                    for tch in range(TCH):
                        pst = p_ps.tile([128, 128], bf16, tag="ps",
                                        name="ps")
                        nc.tensor.transpose(pst[:],
                                            hs[tch][:, 128 * ec:128 * (ec + 1)],
                                            ident[:])
                        if fmt == "bf16":
                            dst = ht[ec][:, 128 * tch:128 * (tch + 1)]
                        else:
                            dst = ht[ec // 2][:, ec % 2,
                                              128 * tch:128 * (tch + 1)]
                        nc.vector.tensor_copy(dst, pst[:])
                return ht

            ht = layer_norm_transposed(xt, ln1g_b, ln1b_b)

            def dump_f32(dst, src_bf16):
                stg = p_htok.tile([128, src_bf16.shape[-1]], f32, tag="dump",
                                  name="dump")
                nc.vector.tensor_copy(stg[:], src_bf16)
                nc.sync.dma_start(dst, stg[:])

            if sn == 1:
                o = dbg_tensor("dbg_ht", [E, T])
                for ec in range(ECH):
                    dump_f32(o[128 * ec:128 * (ec + 1), :], ht[ec][:])

            if sn >= 2:
                # attention-phase pools; closed before the MLP phase
                att_cms = [
                    tc.tile_pool(name="qk", bufs=4),
                    tc.tile_pool(name="vpool", bufs=TCH),
                    tc.tile_pool(name="esc", bufs=8),
                    tc.tile_pool(name="attn", bufs=NPAIR),
                    tc.tile_pool(name="norm", bufs=4),
                ]
                p_qk, p_v, p_esc, p_attn, p_norm = (
                    cm.__enter__() for cm in att_cms)

                # ---- V = h @ wv -> token layout [t, head, 65] + ones col ----
                vt = []
                for tch in range(TCH):
                    v = p_v.tile([128, NH, HD + 1], bf16, tag="v", name="v")
                    nc.vector.tensor_copy(v[:, :, HD:HD + 1], ones16[:])
                    vt.append(v)
                for half in range(2):
                    esl = slice(512 * half, 512 * (half + 1))
                    ys = [p_ps.tile([128, 512], f32, tag="ps", name="ps")
                          for _ in range(TCH)]
                    for ec in range(ECH):
                        wv_t = p_wmov.tile([128, 512], bf16, tag="wmov",
                                           name="wmov")
                        nc.sync.dma_start(wv_t[:],
                                          wv_d[128 * ec:128 * (ec + 1), esl])
                        for tch in range(TCH):
                            nc.tensor.matmul(
                                ys[tch][:],
                                ht[ec][:, 128 * tch:128 * (tch + 1)],
                                wv_t[:], start=(ec == 0),
                                stop=(ec == ECH - 1))
                    for tch in range(TCH):
                        nc.vector.tensor_copy(
                            vt[tch][:, 8 * half:8 * (half + 1), 0:HD],
                            ys[tch][:].rearrange("p (h d) -> p h d", d=HD))

                # ---- per head pair: qT/kT, scores, softmax, att ----
                attn_t = []
                for pair in range(NPAIR if sn >= 3 else 1):
                    wq_t = p_wsta.tile([128, ECH, 128], bf16, tag="wsta",
                                       name="wsta")
                    nc.sync.dma_start(wq_t[:], wq_d[pair])
                    wk_t = p_wsta.tile([128, ECH, 128], bf16, tag="wsta",
                                       name="wsta")
                    nc.sync.dma_start(wk_t[:], wk_d[pair])
                    qT = p_qk.tile([128, T], bf16, tag="qk", name="qk")
                    kT = p_qk.tile([128, T], bf16, tag="qk", name="qk")
                    for (w_t, dst) in ((wq_t, qT), (wk_t, kT)):
                        for th in range(NTQ):
                            tsl = slice(TQW * th, TQW * (th + 1))
                            ps = p_ps.tile([128, 512], f32, tag="ps",
                                           name="ps")
                            for ec in range(ECH):
                                nc.tensor.matmul(
                                    ps[:], w_t[:, ec, :], ht[ec][:, tsl],
                                    start=(ec == 0), stop=(ec == ECH - 1))
                            nc.vector.tensor_copy(dst[:, tsl], ps[:])

                    if sn == 2 and pair == 0:
                        oq = dbg_tensor("dbg_qT", [128, T])
                        dump_f32(oq[:, :], qT[:])
                        ok_ = dbg_tensor("dbg_kT", [128, T])
                        dump_f32(ok_[:, :], kT[:])
                        break

                    att_pair = p_attn.tile([128, T], bf16, tag="attn",
                                           name="attn")
                    attn_t.append(att_pair)
                    # both heads of the pair interleaved: the two score
                    # matmuls (K=64, stationary base_partition 0 / 64 ->
                    # row-groups (0,0)/(64,0)) are emitted back-to-back so
                    # the PE runs them concurrently in different row groups.
                    for bq in range(NTQ):
                        qsl = slice(TQW * bq, TQW * (bq + 1))
                        nbk = min(TCH, 4 * bq + 4)
                        ps_a = [p_ps.tile([128, 512], f32, tag="ps",
                                          name="ps") for _ in range(2)]
                        for bk in range(nbk):
                            d = bk - 4 * bq
                            ets = []
                            pss = []
                            for hp in range(2):
                                rows = slice(HD * hp, HD * (hp + 1))
                                ps_s = p_ps.tile([128, 512], f32, tag="ps",
                                                 name="ps")
                                nc.tensor.matmul(
                                    ps_s[:],
                                    kT[rows, 128 * bk:128 * (bk + 1)],
                                    qT[rows, qsl], start=True, stop=True)
                                pss.append(ps_s)
                            for hp in range(2):
                                ps_s = pss[hp]
                                et = p_esc.tile([128, 512], bf16, tag="esc",
                                                name="esc")
                                if d <= 0:
                                    nc.scalar.activation(et[:], ps_s[:],
                                                         AF.Exp, scale=0.125)
                                else:
                                    nc.vector.tensor_copy(
                                        et[:, 0:128 * d],
                                        zer384[:, 0:128 * d])
                                    nc.scalar.activation(
                                        et[:, 128 * d:512],
                                        ps_s[:, 128 * d:512],
                                        AF.Exp, scale=0.125)
                                if d >= 0:
                                    dsl = slice(128 * d, 128 * (d + 1))
                                    nc.vector.tensor_mul(et[:, dsl],
                                                         et[:, dsl], tri[:])
                                ets.append(et)
                            for hp in range(2):
                                nc.tensor.matmul(
                                    ps_a[hp][0:HD + 1, :],
                                    vt[bk][:, 2 * pair + hp, :],
                                    ets[hp][:], start=(bk == 0),
                                    stop=(bk == nbk - 1))
                        # normalize by the denominator (row HD of ps_a)
                        for hp in range(2):
                            rcp = p_norm.tile([HD + 1, 512], f32, tag="rcp",
                                              name="rcp")
                            nc.vector.reciprocal(rcp[HD:HD + 1, :],
                                                 ps_a[hp][HD:HD + 1, :])
                            bct = p_norm.tile([HD, 512], f32, tag="bct",
                                              name="bct")
                            rsl = rcp[HD:HD + 1, :]
                            rap = list(rsl.ap)
                            rbc = bass.AP(tensor=rsl.tensor, offset=rsl.offset,
                                          ap=[rap[0], [0, HD], rap[1]])
                            nc.gpsimd.dma_start(out=bct[:], in_=rbc)
                            if hp == 0:
                                nc.vector.tensor_mul(att_pair[0:HD, qsl],
                                                     ps_a[hp][0:HD, :],
                                                     bct[:])
                            else:
                                sc = p_norm.tile([HD, 512], bf16,
                                                 tag="oddsc", name="oddsc")
                                nc.vector.tensor_mul(sc[:], ps_a[hp][0:HD, :],
                                                     bct[:])
                                nc.sync.dma_start(att_pair[HD:128, qsl],
                                                  sc[:])

                if sn == 2:
                    o2 = dbg_tensor("dbg_v", [T, NH * (HD + 1)])
                    for tch in range(TCH):
                        dump_f32(o2[128 * tch:128 * (tch + 1), :],
                                 vt[tch][:].rearrange("p h d -> p (h d)"))
                if sn == 3:
                    o = dbg_tensor("dbg_attnT", [E, T])
                    for pr in range(NPAIR):
                        dump_f32(o[128 * pr:128 * (pr + 1), :],
                                 attn_t[pr][:])

                # ---- out = x + attnT^T @ w_proj (+ b_proj), in-place xt ----
                if sn >= 4:
                    for eo in range(2):
                        esl = slice(512 * eo, 512 * (eo + 1))
                        ys = [p_ps.tile([128, 512], f32, tag="ps", name="ps")
                              for _ in range(TCH)]
                        for pair in range(NPAIR):
                            wp_t = p_wmov.tile([128, 512], bf16, tag="wmov",
                                               name="wmov")
                            nc.sync.dma_start(
                                wp_t[:],
                                wp_d[128 * pair:128 * (pair + 1), esl])
                            for tch in range(TCH):
                                nc.tensor.matmul(
                                    ys[tch][:],
                                    attn_t[pair][:, 128 * tch:128 * (tch + 1)],
                                    wp_t[:], start=(pair == 0),
                                    stop=(pair == NPAIR - 1))
                        for tch in range(TCH):
                            nc.vector.tensor_add(xt[tch][:, esl], ys[tch][:],
                                                 xt[tch][:, esl])
                            if flags["b_proj"]:
                                nc.vector.tensor_add(xt[tch][:, esl],
                                                     xt[tch][:, esl],
                                                     bp_b[:, esl])

                for cm in reversed(att_cms):
                    cm.__exit__(None, None, None)

            if sn == 4:
                o = dbg_tensor("dbg_out1", [T, E])
                for tch in range(TCH):
                    nc.sync.dma_start(o[128 * tch:128 * (tch + 1), :],
                                      xt[tch][:])

            if sn >= 5:
                # ---- LN2 + transpose ----
                h2t = layer_norm_transposed(
                    xt, ln2g_b, ln2b_b, fmt=("fp8" if FP8_MLP else "bf16"))

                # ---- MLP: uT = w1^T h2T; g = gelu(uT+b1); y = g^T w2 ----
                # w1 is DMA'd into SBUF once and reused by both 512-token
                # passes; w2 streams once per pass.
                DR = mybir.MatmulPerfMode.DoubleRow if FP8_MLP else None
                with (tc.tile_pool(name="w1sta", bufs=FCH) as p_w1,
                      tc.tile_pool(name="gt", bufs=FCH) as p_gt):
                    w1_t = []
                    for fc in range(FCH):
                        w = p_w1.tile([128, ECH, 128],
                                      mybir.dt.float8e4 if FP8_MLP else bf16,
                                      tag="w1", name="w1")
                        nc.sync.dma_start(w[:], w1_d[fc])
                        w1_t.append(w)
                    for th in range(NTQ):
                        tsl = slice(TQW * th, TQW * (th + 1))
                        gt = []
                        if FP8_MLP:
                            gt = [p_gt.tile([128, 2, 512], mybir.dt.float8e4,
                                            tag="gt", name="gt")
                                  for _ in range(FCH // 2)]
                        for fc in range(FCH):
                            ps = p_ps.tile([128, 512], f32, tag="ps",
                                           name="ps")
                            if FP8_MLP:
                                for j in range(ECH // 2):
                                    nc.tensor.matmul(
                                        ps[:], w1_t[fc][:, 2 * j:2 * j + 2, :],
                                        h2t[j][:, :, tsl],
                                        start=(j == 0),
                                        stop=(j == ECH // 2 - 1),
                                        perf_mode=DR)
                                nc.scalar.activation(
                                    gt[fc // 2][:, fc % 2, :], ps[:], AF.Gelu,
                                    bias=b1c[:, fc:fc + 1],
                                    scale=1.0 / W1_SCALE)
                            else:
                                for ec in range(ECH):
                                    nc.tensor.matmul(
                                        ps[:], w1_t[fc][:, ec, :],
                                        h2t[ec][:, tsl],
                                        start=(ec == 0), stop=(ec == ECH - 1))
                                g = p_gt.tile([128, 512], bf16, tag="gt",
                                              name="gt")
                                nc.scalar.activation(g[:], ps[:], AF.Gelu,
                                                     bias=b1c[:, fc:fc + 1])
                                gt.append(g)
                        for eo in range(2):
                            esl = slice(512 * eo, 512 * (eo + 1))
                            ys = [p_ps.tile([128, 512], f32, tag="ps",
                                            name="ps") for _ in range(4)]
                            if FP8_MLP:
                                for j2 in range(FF // 256):
                                    w2_t = p_wmov.tile(
                                        [128, 2, 512], mybir.dt.float8e4,
                                        tag="wmov8", name="wmov8")
                                    nc.sync.dma_start(w2_t[:],
                                                      w2_d[eo, j2])
                                    for tl in range(4):
                                        nc.tensor.matmul(
                                            ys[tl][:],
                                            gt[j2][:, :,
                                                   128 * tl:128 * (tl + 1)],
                                            w2_t[:], start=(j2 == 0),
                                            stop=(j2 == FF // 256 - 1),
                                            perf_mode=DR)
                            else:
                                for fc in range(FCH):
                                    w2_t = p_wmov.tile([128, 512], bf16,
                                                       tag="wmov",
                                                       name="wmov")
                                    nc.sync.dma_start(
                                        w2_t[:],
                                        w2_d[128 * fc:128 * (fc + 1), esl])
                                    for tl in range(4):
                                        nc.tensor.matmul(
                                            ys[tl][:],
                                            gt[fc][:,
                                                   128 * tl:128 * (tl + 1)],
                                            w2_t[:], start=(fc == 0),
                                            stop=(fc == FCH - 1))
                            for tl in range(4):
                                tch = 4 * th + tl
                                if FP8_MLP:
                                    ydsc = p_htok.tile([128, 512], f32,
                                                       tag="ydsc",
                                                       name="ydsc", bufs=4)
                                    nc.scalar.activation(
                                        ydsc[:], ys[tl][:], AF.Copy,
                                        scale=1.0 / W2_SCALE)
                                    nc.vector.tensor_add(xt[tch][:, esl],
                                                         ydsc[:],
                                                         xt[tch][:, esl])
                                else:
                                    nc.vector.tensor_add(xt[tch][:, esl],
                                                         ys[tl][:],
                                                         xt[tch][:, esl])
                                if flags["b2"]:
                                    nc.vector.tensor_add(xt[tch][:, esl],
                                                         xt[tch][:, esl],
                                                         b2_b[:, esl])
                        for tl in range(4):
                            tch = 4 * th + tl
                            nc.sync.dma_start(
                                out_d[128 * tch:128 * (tch + 1), :],
                                xt[tch][:])

    nc.compile()
    return nc, dbg_outs


_CACHE = {}


def _get_program(flags_key, stage="full", loop_n=1):
    key = (flags_key, stage, loop_n)
    if key not in _CACHE:
        flags = dict(zip(("ln1_gb", "ln2_gb", "b_proj", "b2"), flags_key))
        _CACHE[key] = _build_program(flags, stage, loop_n=loop_n)
    return _CACHE[key]


def _flags_for(inputs):
    return {
        "ln1_gb": not (np.all(np.asarray(inputs["ln1_g"]) == 1.0)
                       and np.all(np.asarray(inputs["ln1_b"]) == 0.0)),
        "ln2_gb": not (np.all(np.asarray(inputs["ln2_g"]) == 1.0)
                       and np.all(np.asarray(inputs["ln2_b"]) == 0.0)),
        "b_proj": not np.all(np.asarray(inputs["b_proj"]) == 0.0),
        "b2": not np.all(np.asarray(inputs["b2"]) == 0.0),
    }


def _make_in_maps(inputs, flags, cores):
    from concourse import mybir

    bf16 = mybir.dt.np(mybir.dt.bfloat16)
    common = {}
    # stationary-tile pre-tiling: [e_in, n_out] -> [tile, p, chunk, n] with
    # e_in = chunk*128 + p and n_out = tile*128 + n
    for name, ntile in (("wq", NPAIR), ("wk", NPAIR), ("w1", FCH)):
        w = np.asarray(inputs[name], np.float32)
        w = w.reshape(ECH, 128, ntile, 128).transpose(2, 1, 0, 3)
        common[name] = np.ascontiguousarray(w).astype(bf16)
    for name in ("wv", "w_proj", "w2"):
        common[name] = np.ascontiguousarray(
            np.asarray(inputs[name], np.float32)).astype(bf16)
    if FP8_MLP:
        fp8 = mybir.dt.np(mybir.dt.float8e4)
        w1 = np.asarray(inputs["w1"], np.float32)
        w1 = w1.reshape(ECH, 128, FCH, 128).transpose(2, 1, 0, 3) * W1_SCALE
        common["w1"] = np.ascontiguousarray(w1).astype(fp8)
        # w2 [FF, E] -> [eo, j2, p, sub, n] with f = j2*256 + sub*128 + p
        # and e = eo*512 + n, so each [128, 2, 512] moving tile is a fully
        # contiguous per-partition DMA.
        w2 = np.asarray(inputs["w2"], np.float32)
        w2 = w2.reshape(FF // 256, 2, 128, 2, E // 2)
        w2 = w2.transpose(3, 0, 2, 1, 4) * W2_SCALE
        common["w2"] = np.ascontiguousarray(w2).astype(fp8)
    # b1 [FF] -> [p, fc] with f = fc*128 + p
    common["b1"] = np.ascontiguousarray(
        np.asarray(inputs["b1"], np.float32).reshape(FCH, 128).T)
    for name, flag in (("ln1_g", "ln1_gb"), ("ln1_b", "ln1_gb"),
                       ("ln2_g", "ln2_gb"), ("ln2_b", "ln2_gb"),
                       ("b_proj", "b_proj"), ("b2", "b2")):
        if flags[flag]:
            common[name] = np.ascontiguousarray(inputs[name], np.float32)
    x = np.ascontiguousarray(inputs["x"], np.float32)
    return [{"x": x[c], **common} for c in cores]


def _run(inputs, stage="full", cores=None):
    from concourse.bass_utils import run_bass_kernel_spmd

    if cores is None:
        cores = list(range(NCORES))
    flags = _flags_for(inputs)
    flags_key = tuple(flags[k] for k in ("ln1_gb", "ln2_gb", "b_proj", "b2"))
    nc, dbg = _get_program(flags_key, stage)
    in_maps = _make_in_maps(inputs, flags, cores)
    res = run_bass_kernel_spmd(nc, in_maps, cores)
    return res, dbg


_FN_CACHE = {}


def kernel(**inputs) -> np.ndarray:
    import jax

    flags = _flags_for(inputs)
    flags_key = tuple(flags[k] for k in ("ln1_gb", "ln2_gb", "b_proj", "b2"))
    nc, _ = _get_program(flags_key, "full")
    if id(nc) not in _FN_CACHE:
        _FN_CACHE[id(nc)] = _make_jit_fn(nc)
    fn, mesh, in_names, out_names, out_avals = _FN_CACHE[id(nc)]
    in_maps = _make_in_maps(inputs, flags, list(range(NCORES)))
    dev_args = _device_args(in_maps, in_names, out_avals, mesh)
    out = fn(*dev_args)
    jax.block_until_ready(out)
    oi = out_names.index("out")
    return np.asarray(out[oi]).reshape(NCORES, T, E)


def _make_jit_fn(nc):
    """Build a jitted SPMD executor for a compiled Bass program.

    Returns (fn, mesh, in_names, out_names, out_avals)."""
    import jax
    from jax.experimental.shard_map import shard_map
    from jax.sharding import Mesh, PartitionSpec
    from concourse import mybir
    from concourse.bass2jax import (_bass_exec_p, install_neuronx_cc_hook,
                                    partition_id_tensor)

    install_neuronx_cc_hook()
    partition_name = (nc.partition_id_tensor.name
                      if nc.partition_id_tensor else None)
    in_names, out_names, out_avals = [], [], []
    for alloc in nc.m.functions[0].allocations:
        if not isinstance(alloc, mybir.MemoryLocationSet):
            continue
        name = alloc.memorylocations[0].name
        if alloc.kind == "ExternalInput":
            if name != partition_name:
                in_names.append(name)
        elif alloc.kind == "ExternalOutput":
            out_names.append(name)
            shape = tuple(alloc.tensor_shape)
            dtype = mybir.dt.np(alloc.dtype)
            out_avals.append(jax.core.ShapedArray(shape, dtype))
    n_params = len(in_names)
    all_names = in_names + out_names
    if partition_name is not None:
        all_names = all_names + [partition_name]

    def _body(*args):
        operands = list(args)
        if partition_name is not None:
            operands.append(partition_id_tensor())
        outs = _bass_exec_p.bind(
            *operands,
            out_avals=tuple(out_avals),
            in_names=tuple(all_names),
            out_names=tuple(out_names),
            lowering_input_output_aliases=(),
            sim_require_finite=True,
            sim_require_nnan=True,
            nc=nc,
        )
        return tuple(outs)

    devices = jax.devices()[:NCORES]
    mesh = Mesh(np.asarray(devices), ("core",))
    n_outs = len(out_names)
    in_specs = (PartitionSpec("core"),) * (n_params + n_outs)
    out_specs = (PartitionSpec("core"),) * n_outs
    fn = jax.jit(shard_map(_body, mesh=mesh, in_specs=in_specs,
                           out_specs=out_specs, check_rep=False),
                 keep_unused=True)
    return fn, mesh, in_names, out_names, out_avals


def _device_args(in_maps, in_names, out_avals, mesh):
    """device_put the per-core inputs pre-sharded across the mesh (one hop,
    no per-call resharding) plus zeroed output buffers."""
    import jax
    from jax.sharding import NamedSharding, PartitionSpec

    sharding = NamedSharding(mesh, PartitionSpec("core"))
    concat_in = [
        np.concatenate([np.asarray(in_maps[c][nm]) for c in range(NCORES)],
                       axis=0)
        for nm in in_names
    ]
    concat_zeros = [
        np.zeros((NCORES * a.shape[0], *a.shape[1:]), a.dtype)
        for a in out_avals
    ]
    return [jax.device_put(a, sharding) for a in concat_in + concat_zeros]


def _timed_run(inputs, iters=10, stage="full"):
    """Measure per-application device time of the kernel.

    Two variants of the same program are run: one computing the block once,
    and one recomputing it LOOP_N times in an on-device For_i loop (every
    iteration reads the same DRAM inputs and rewrites the same output, so
    the result is identical).  Inputs are device-resident and pre-sharded;
    the marginal time (t_loop - t_single) / (LOOP_N - 1), min over `iters`
    wall-clock reps of each, is the per-iteration hardware execution time
    with the fixed dispatch/tunnel overhead cancelled out.

    Returns (out [B,T,E], seconds_per_iter).
    """
    import time
    import jax

    LOOP_N = 41
    flags = _flags_for(inputs)
    flags_key = tuple(flags[k] for k in ("ln1_gb", "ln2_gb", "b_proj", "b2"))
    nc1, _ = _get_program(flags_key, stage)
    ncN, _ = _get_program(flags_key, stage, loop_n=LOOP_N)
    in_maps = _make_in_maps(inputs, flags, list(range(NCORES)))

    fn1, mesh, in_names, out_names, out_avals = _make_jit_fn(nc1)
    fnN, _, _, _, _ = _make_jit_fn(ncN)
    dev_args = _device_args(in_maps, in_names, out_avals, mesh)

    out1 = fn1(*dev_args)
    jax.block_until_ready(out1)     # warm-up (compile + first run)
    outN = fnN(*dev_args)
    jax.block_until_ready(outN)
    # Interleaved single/loop calls; congestion spikes in the tunnel or on
    # the device are strictly additive, so min-over-reps of each estimates
    # the uncongested time and their difference the marginal per-iteration
    # hardware time.
    t_single, t_loop = [], []
    for _ in range(max(iters, 20)):
        t0 = time.perf_counter()
        out1 = fn1(*dev_args)
        jax.block_until_ready(out1)
        t1 = time.perf_counter()
        outN = fnN(*dev_args)
        jax.block_until_ready(outN)
        t2 = time.perf_counter()
        t_single.append(t1 - t0)
        t_loop.append(t2 - t1)
    per_iter = max(0.0, min(t_loop) - min(t_single)) / (LOOP_N - 1)
    oi = out_names.index("out")
    res = np.asarray(outN[oi]).reshape(NCORES, T, E)
    return res, per_iter

